# revision 35
# baseline (speedup 1.0000x reference)
"""GQA multi-head self-attention (16 heads / 4 KV heads / head_dim 128) with
rotate-half RoPE, for B=2, S=2048, E=2048 fp32 inputs, on 8 NeuronCores.

Sharding: 8 cores = 2 batches x 4 tensor-parallel ranks. Each rank owns 4
query heads + 1 KV head (column slices of Wq/Wk/Wv) and the matching row
slice of Wo; per-rank partial outputs are summed on the host (the Wo
all-reduce), batches are concatenated.

v4 (bf16 + scheduling): all matmul operands bf16 (PSUM stays fp32) -- keeps
the PE at 1 col/cycle, enables fast weight loads, halves HBM traffic, and
runs diagonal score blocks at N=128 full rate. Rel err ~6e-3 (budget 2e-2).

Softmax row-sums avoid PE matmuls per block: exp blocks accumulate
elementwise into two SBUF accumulators (even blocks on DVE, odd on the
otherwise-idle GpSimd; the last block on DVE to shorten the end-of-head
chain), then one ones-matmul per accumulator reduces partitions.

Scheduling: the attention q-blocks run ascending, and every head carries
dependency-free PE filler so exp/softmax chain latency never idles the PE:
the g=3 Q projections are withheld from phase A and emitted one chain per
g=0 head (whose attention is tiny), and each completed q-block's output
projection is dripped between later score blocks with an allowance that
spreads it across the following q-block's heads. The final q-block's output
projection is split: heads 0-2 partials stream to `out` early, head 3's
contribution lands in `out2` and is added on the host, shrinking the
end-of-kernel drain.
"""

import sys

sys.path.insert(0, "/opt/trn_rl_repo")

from contextlib import ExitStack

import ml_dtypes
import numpy as np

import concourse.bacc as bacc
import concourse.tile as tile
from concourse import mybir
from concourse.bass_utils import run_bass_kernel_spmd

BF = mybir.dt.bfloat16
F32 = mybir.dt.float32
NPBF = ml_dtypes.bfloat16

S = 2048  # sequence length
E = 2048  # embed dim
D = 128  # head dim
HQ = 4  # query heads per core
SB = 512  # s-block (free-dim tile)
NSB = S // SB  # 4
NEC = E // D  # 16 contraction chunks
NSC = S // D  # 16 s-chunks
SCALE = 1.0 / float(np.sqrt(D))

_CACHED_NC = None


def _build_nc():
    nc = bacc.Bacc("TRN2", target_bir_lowering=False, debug=False)

    xT = nc.dram_tensor("xT", [NSB, 4, D, NEC // 4, SB], BF, kind="ExternalInput")
    wq = nc.dram_tensor("wq", [HQ, 2, D, NEC // 2, D], BF, kind="ExternalInput")
    wk = nc.dram_tensor("wk", [D, NEC, D], BF, kind="ExternalInput")
    wv = nc.dram_tensor("wv", [D, NEC, D], BF, kind="ExternalInput")
    wo = nc.dram_tensor("wo", [D, HQ, E], BF, kind="ExternalInput")
    cosT = nc.dram_tensor("cosT", [D, S], BF, kind="ExternalInput")
    sinT = nc.dram_tensor("sinT", [D, S], BF, kind="ExternalInput")
    rot = nc.dram_tensor("rot", [D, D], BF, kind="ExternalInput")
    ident = nc.dram_tensor("ident", [D, D], BF, kind="ExternalInput")
    onesc = nc.dram_tensor("onesc", [D, D], BF, kind="ExternalInput")
    masks = nc.dram_tensor("masks", [D, 4, SB], BF, kind="ExternalInput")
    out = nc.dram_tensor("out", [S, E], BF, kind="ExternalOutput")
    out2 = nc.dram_tensor("out2", [SB, E], BF, kind="ExternalOutput")

    with tile.TileContext(nc) as tc, ExitStack() as ctx:
        pers = ctx.enter_context(tc.tile_pool(name="pers", bufs=1))
        qts = [
            [
                pers.tile([D, SB], BF, tag=f"qt{h}_{g}", name=f"qt{h}_{g}")
                for g in range(NSB)
            ]
            for h in range(HQ)
        ]
        kts = [
            pers.tile([D, SB], BF, tag=f"kts{g}", name=f"kts{g}")
            for g in range(NSB)
        ]
        vsb = [
            pers.tile([D, SB // D, D], BF, tag=f"vsb{g}", name=f"vsb{g}")
            for g in range(NSB)
        ]

        ps_pool = ctx.enter_context(tc.tile_pool(name="ps", bufs=1, space="PSUM"))

        class _TagPool:
            def __init__(self, tag, bufs):
                self.tag, self.bufs, self.n = tag, bufs, 0

            def tile(self, shape, dtype, **kw):
                self.n += 1
                return ps_pool.tile(
                    shape, dtype, tag=self.tag, bufs=self.bufs,
                    name=f"{self.tag}_{self.n}",
                )

        psq_pool = pst_pool = _TagPool("st3", 3)
        pskv_pool = psa_pool = _TagPool("acc", 2)
        psr_pool = psl_pool = _TagPool("one", 1)
        pstr_pool = pso_pool = _TagPool("sm", 2)

        # persistent phase-B constants, DMA'd during phase A so the A->B
        # boundary has no DMA wait
        wo_pool = ctx.enter_context(tc.tile_pool(name="woP", bufs=1))
        wot = wo_pool.tile([D, HQ, E], BF)
        lin_pool = ctx.enter_context(tc.tile_pool(name="lin", bufs=1))
        onest = lin_pool.tile([D, D], BF, tag="onest")
        maskt = lin_pool.tile([D, 4, SB], BF, tag="maskt")

        # pools that phase A and the deferred g=3 Q chains both use
        xs_pool = ctx.enter_context(tc.tile_pool(name="xs", bufs=8))
        wA_pool = ctx.enter_context(tc.tile_pool(name="wA", bufs=1))
        ropet = ctx.enter_context(tc.tile_pool(name="ropet", bufs=2))

        # PE pre-warm: dependency-free dummy matmuls on never-written SBUF
        # keep the PE busy through the input DMA ramp so the HAM clock-gate
        # un-throttles before the first real chain and the ramp is hidden.
        dummy_pool = ctx.enter_context(tc.tile_pool(name="dummy", bufs=1))
        dumt = dummy_pool.tile([D, SB], BF, tag="dumt")
        nc.gpsimd.memset(dumt[:], 0.0)
        psd = psr_pool.tile([D, SB], F32)
        for _ in range(12):
            nc.tensor.matmul(psd[:], dumt[:, 0:D], dumt[:], start=True, stop=True)

        # ---- Phase A: QKV projections + RoPE + V transpose ----
        # First DMAs in: part of the K weights plus the first g=0 x tile so
        # the first matmul chain starts as early as possible, then the rest
        # of the x stream, tables and later weights.
        def load_x(g):
            tiles = []
            for qt in range(4):
                t = xs_pool.tile(
                    [D, NEC // 4, SB], BF, tag="xs", name=f"xs{g}_{qt}"
                )
                nc.sync.dma_start(t[:], xT[g, qt])
                tiles.append(t)
            return tiles

        def load_wq(h):
            halves = []
            for hf in range(2):
                t = wA_pool.tile(
                    [D, NEC // 2, D], BF, tag=f"wq{h}_{hf}", name=f"wq{h}_{hf}"
                )
                nc.sync.dma_start(t[:], wq[h, hf])
                halves.append(t)
            return halves

        wkt = wA_pool.tile([D, NEC, D], BF)
        nc.sync.dma_start(wkt[:, 0:4, :], wk[:, 0:4, :])
        xh0 = []
        t = xs_pool.tile([D, NEC // 4, SB], BF, tag="xs", name="xs0_0")
        nc.sync.dma_start(t[:], xT[0, 0])
        xh0.append(t)
        nc.sync.dma_start(wkt[:, 4:, :], wk[:, 4:, :])
        for qt in range(1, 4):
            t = xs_pool.tile([D, NEC // 4, SB], BF, tag="xs", name=f"xs0_{qt}")
            nc.sync.dma_start(t[:], xT[0, qt])
            xh0.append(t)
        wvt = wA_pool.tile([D, NEC, D], BF)
        nc.sync.dma_start(wvt[:], wv[:])
        xh1 = load_x(1)  # prefetch g=1's x so its chains never wait
        rott = wA_pool.tile([D, D], BF, tag="rott")
        nc.sync.dma_start(rott[:], rot[:])
        cost = wA_pool.tile([D, S], BF, tag="cost")
        nc.sync.dma_start(cost[:], cosT[:])
        sint = wA_pool.tile([D, S], BF, tag="sint")
        nc.sync.dma_start(sint[:], sinT[:])
        wqh = [load_wq(h) for h in range(HQ)]
        idt = wA_pool.tile([D, D], BF, tag="idt")
        nc.sync.dma_start(idt[:], ident[:])
        # phase-B constants, early
        nc.sync.dma_start(onest[:], onesc[:])
        nc.sync.dma_start(maskt[:], masks[:])
        nc.sync.dma_start(wot[:], wo[:])

        def rope_store(src_ps, dst_slice, scale):
            # qc = rounded bf16 copy of the projection (folds 1/sqrt(D))
            qc = ropet.tile([D, SB], BF, tag="qc")
            nc.scalar.activation(
                qc[:], src_ps[:], mybir.ActivationFunctionType.Copy,
                scale=scale,
            )
            # pr = signed rotate-half via PE permutation matmul
            pr = psr_pool.tile([D, SB], F32)
            nc.tensor.matmul(pr[:], rott[:], qc[:], start=True, stop=True)
            tm = ropet.tile([D, SB], BF, tag="tm")
            nc.vector.tensor_mul(tm[:], qc[:], cost[:, dst_slice])
            tr = ropet.tile([D, SB], F32, tag="tr")
            nc.vector.tensor_mul(tr[:], pr[:], sint[:, dst_slice])
            return qc, tm, tr

        def emit_q(h, g, xh):
            def xc(e):
                return xh[e // (NEC // 4)][:, e % (NEC // 4), :]

            psq = psq_pool.tile([D, SB], F32)
            for e in range(NEC):
                nc.tensor.matmul(
                    psq[:],
                    wqh[h][e // (NEC // 2)][:, e % (NEC // 2), :],
                    xc(e),
                    start=(e == 0),
                    stop=(e == NEC - 1),
                )
            _, tm, tr = rope_store(psq, slice(g * SB, (g + 1) * SB), SCALE)
            nc.vector.tensor_add(qts[h][g][:], tm[:], tr[:])

        xh3 = None
        for g in range(NSB):
            gsl = slice(g * SB, (g + 1) * SB)
            xh = (xh0, xh1, None, None)[g] if g < 2 else load_x(g)

            def xc(e):
                return xh[e // (NEC // 4)][:, e % (NEC // 4), :]

            psk = pskv_pool.tile([D, SB], F32)
            for e in range(NEC):
                nc.tensor.matmul(
                    psk[:], wkt[:, e, :], xc(e),
                    start=(e == 0), stop=(e == NEC - 1),
                )
            _, tm, tr = rope_store(psk, gsl, 1.0)
            nc.vector.tensor_add(kts[g][:], tm[:], tr[:])

            psv = pskv_pool.tile([D, SB], F32)
            for e in range(NEC):
                nc.tensor.matmul(
                    psv[:], wvt[:, e, :], xc(e),
                    start=(e == 0), stop=(e == NEC - 1),
                )
            vt = ropet.tile([D, SB], BF, tag="vt")
            nc.vector.tensor_copy(vt[:], psv[:])
            for c in range(SB // D):
                ptr = pstr_pool.tile([D, D], BF)
                nc.tensor.transpose(ptr[:], vt[:, c * D : (c + 1) * D], idt[:])
                nc.vector.tensor_copy(vsb[g][:, c, :], ptr[:])

            if g == NSB - 1:
                # withhold the g=3 Q projections: they become PE filler for
                # the filler-less g=0 attention heads
                xh3 = xh
            else:
                for h in range(HQ):
                    emit_q(h, g, xh)

        # ---- Phase B: attention (scores^T -> exp -> mask -> l, attn^T) ----
        atn_pool = ctx.enter_context(tc.tile_pool(name="atnP", bufs=1))
        atn = [
            [
                atn_pool.tile([D, SB], BF, tag=f"atn{h}_{g}", name=f"atn{h}_{g}")
                for g in range(NSB)
            ]
            for h in range(HQ)
        ]
        with (
            tc.tile_pool(name="ptp", bufs=4) as pt_pool,
            tc.tile_pool(name="accp", bufs=2) as acc_pool,
            tc.tile_pool(name="lbp", bufs=2) as lb_pool,
            tc.tile_pool(name="outs", bufs=8) as out_pool,
        ):
            # Output-projection work for one (sc, nb) pair over heads `hs`:
            # emitted as filler between attention blocks so these
            # dependency-free matmuls soak up PE bubbles while exp/mask
            # chains are in flight. Every 4th copy goes to the Scalar engine
            # to keep DVE below the PE's phase-B load.
            nco = [0]

            def emit_c(sc, nb, hs=tuple(range(HQ)), dst=None, row0=None,
                       copy_eng=None, pool=None):
                po = (pool or pso_pool).tile([D, SB], F32)
                for i, h in enumerate(hs):
                    nc.tensor.matmul(
                        po[:],
                        atn[h][sc // 4][:, (sc % 4) * D : (sc % 4 + 1) * D],
                        wot[:, h, nb * SB : (nb + 1) * SB],
                        start=(i == 0),
                        stop=(i == len(hs) - 1),
                    )
                nco[0] += 1
                ot = out_pool.tile([D, SB], BF, tag="ot", name=f"ot{nco[0]}")
                if copy_eng is None:
                    copy_eng = "s" if nco[0] % 4 == 0 else "v"
                if copy_eng == "s":
                    nc.scalar.copy(ot[:], po[:])
                else:
                    nc.vector.tensor_copy(ot[:], po[:])
                if dst is None:
                    dst, row0 = out, sc * D
                nc.sync.dma_start(
                    dst[row0 : row0 + D, nb * SB : (nb + 1) * SB], ot[:]
                )

            cqueue = []
            for g in range(NSB):
                nkb = 4 * (g + 1)
                for h in range(HQ):
                    last_head = g == NSB - 1 and h == HQ - 1
                    # spread remaining filler over this q-block's heads
                    allow = 0
                    if cqueue and not last_head:
                        hleft = HQ - h if g < NSB - 1 else HQ - 1 - h
                        allow = -(-len(cqueue) // max(hleft, 1))
                    dripped = 0
                    # g=0 heads have no output-projection filler yet: the
                    # withheld g=3 Q projection chain (on the otherwise-idle
                    # sm PSUM tag) interleaves with the score blocks instead
                    psq3 = pso_pool.tile([D, SB], F32) if g == 0 else None
                    pa = psa_pool.tile([D, SB], F32)
                    # dual softmax-denominator accumulators: even score blocks
                    # sum on DVE, odd blocks on GpSimd (except the last block,
                    # which goes to DVE/accA to shorten the end-of-head chain)
                    accA = acc_pool.tile([D, SB], BF, tag="accA")
                    accB = None
                    if g > 0:
                        accB = acc_pool.tile(
                            [D, SB], BF, tag="accB", name="accB"
                        )
                    pending = []

                    def consume(kb, pt, qo):
                        nc.tensor.matmul(
                            pa[:, qo:SB], vsb[kb // 4][:, kb % 4, :], pt[:, qo:SB],
                            start=(kb == 0), stop=(kb == nkb - 1),
                        )

                    for kb in range(nkb):
                        # Diagonal blocks: queries below kb*D are fully
                        # masked; shrink N to the live 128-multiple range.
                        r = kb - 4 * g
                        qo = 0 if r < 1 else r * D
                        ps = pst_pool.tile([D, SB], F32)
                        nc.tensor.matmul(
                            ps[:, qo:SB],
                            kts[kb // 4][:, (kb % 4) * D : (kb % 4 + 1) * D],
                            qts[h][g][:, qo:SB],
                            start=True,
                            stop=True,
                        )
                        pt = pt_pool.tile([D, SB], BF, tag="pt")
                        nc.scalar.activation(
                            pt[:, qo:SB], ps[:, qo:SB],
                            mybir.ActivationFunctionType.Exp,
                        )
                        if r >= 0:
                            # the causal mask is non-trivial only on the
                            # 128-wide diagonal query sub-tile; columns past
                            # it are all-ones, columns before are excluded
                            # by qo
                            dsl = slice(r * D, (r + 1) * D)
                            nc.vector.tensor_mul(
                                pt[:, dsl], pt[:, dsl], maskt[:, r, dsl]
                            )
                        # even blocks and the last two odd blocks sum on DVE
                        # (fast, so the denominator closes right after the
                        # exp pipeline); GpSimd takes only early odd blocks,
                        # finishing well before the head's pl matmul needs it
                        if kb == 0:
                            nc.vector.tensor_copy(accA[:], pt[:])
                        elif kb % 2 == 1 and kb <= nkb - 5:
                            if kb == 1:
                                nc.gpsimd.tensor_copy(accB[:], pt[:])
                            else:
                                nc.gpsimd.tensor_add(
                                    accB[:, qo:SB], accB[:, qo:SB], pt[:, qo:SB]
                                )
                        else:
                            nc.vector.tensor_add(
                                accA[:, qo:SB], accA[:, qo:SB], pt[:, qo:SB]
                            )
                        pending.append((kb, pt, qo))
                        # keep PE two score-blocks ahead of the exp pipeline
                        if len(pending) > 2:
                            consume(*pending.pop(0))
                        # drip output-projection filler between score blocks
                        # (one group held back for the head's drain)
                        if kb % 2 == 1 and dripped + 1 < allow and cqueue:
                            emit_c(*cqueue.pop(0))
                            dripped += 1
                        if last_head and kb % 2 == 1 and kb <= 11:
                            # stream heads 0-2 of the final q-block's output
                            # projection while head 3 finishes (sc 15 is
                            # held back to cover the head's drain)
                            i = kb // 2
                            emit_c(12 + i // 2, 2 * (i % 2), hs=(0, 1, 2),
                                   copy_eng="s")
                            emit_c(12 + i // 2, 2 * (i % 2) + 1, hs=(0, 1, 2),
                                   copy_eng="s")
                        if g == 0:
                            for e in range(kb * 4, kb * 4 + 4):
                                nc.tensor.matmul(
                                    psq3[:],
                                    wqh[h][e // (NEC // 2)][:, e % (NEC // 2), :],
                                    xh3[e // (NEC // 4)][:, e % (NEC // 4), :],
                                    start=(e == 0),
                                    stop=(e == NEC - 1),
                                )
                    ph = [(15, nb) for nb in range(4)] if last_head else []
                    for j, item in enumerate(pending):
                        if j == 1:
                            # filler emitted BEFORE the waiting pv matmul so
                            # the in-order PE has work while exp/mask drain
                            if last_head:
                                emit_c(*ph.pop(0), hs=(0, 1, 2), copy_eng="s")
                                emit_c(*ph.pop(0), hs=(0, 1, 2), copy_eng="s")
                            elif dripped < allow and cqueue:
                                emit_c(*cqueue.pop(0))
                                dripped += 1
                        consume(*item)

                    # softmax denominator: reduce both accumulators'
                    # partitions with ones-matmuls (early-finishing
                    # accumulator first), then normalize
                    pl = psl_pool.tile([D, SB], F32)
                    if g > 0:
                        nc.tensor.matmul(
                            pl[:], onest[:], accB[:], start=True, stop=False
                        )
                        nc.tensor.matmul(
                            pl[:], onest[:], accA[:], start=False, stop=True
                        )
                    else:
                        nc.tensor.matmul(
                            pl[:], onest[:], accA[:], start=True, stop=True
                        )
                    while ph:
                        emit_c(*ph.pop(0), hs=(0, 1, 2), copy_eng="s")
                    lb = lb_pool.tile([D, SB], F32, tag="lb")
                    nc.vector.reciprocal_approx_fast(lb[:], pl[:])
                    nc.vector.tensor_mul(atn[h][g][:], pa[:], lb[:])

                    if g == 0:
                        # finish the interleaved g=3 Q chain: RoPE + store
                        _, tm3, tr3 = rope_store(
                            psq3, slice((NSB - 1) * SB, NSB * SB), SCALE
                        )
                        nc.vector.tensor_add(qts[h][NSB - 1][:], tm3[:], tr3[:])
                    else:
                        while dripped < allow and cqueue:
                            emit_c(*cqueue.pop(0))
                            dripped += 1
                if g < NSB - 1:
                    cqueue.extend(
                        (sc, nb)
                        for sc in range(4 * g, 4 * (g + 1))
                        for nb in range(E // SB)
                    )
            # any leftover filler
            while cqueue:
                emit_c(*cqueue.pop(0))
            # head 3 of the final q-block: single-matmul groups into out2,
            # copies alternating scalar/vector so the drain isn't DVE-bound
            # attention is over, so the score-pipeline PSUM pool is free:
            # alternating the singles' accumulators between the sm and st3
            # pools deepens the rotation and makes the drain copy-bound
            for i, (sc, nb) in enumerate(
                (sc, nb) for sc in range(12, 16) for nb in range(E // SB)
            ):
                emit_c(sc, nb, hs=(3,), dst=out2, row0=(sc - 12) * D,
                       copy_eng=("s" if i % 2 == 0 else "v"),
                       pool=(pst_pool if i % 2 == 0 else pso_pool))

    nc.finalize()
    return nc


def _get_nc():
    global _CACHED_NC
    if _CACHED_NC is None:
        _CACHED_NC = _build_nc()
    return _CACHED_NC


def _host_tables():
    inv_freq = 1.0 / (10000.0 ** (np.arange(0, D, 2, dtype=np.float64) / D))
    ang = np.arange(S, dtype=np.float64)[:, None] * inv_freq[None, :]  # [S, 64]
    cos_half = np.cos(ang).T.astype(np.float32)  # [64, S]
    sin_half = np.sin(ang).T.astype(np.float32)
    cosT = np.concatenate([cos_half, cos_half], axis=0)  # [128, S]
    sinT = np.concatenate([sin_half, sin_half], axis=0)

    rot = np.zeros((D, D), dtype=np.float32)  # lhsT of rotate-half
    half = D // 2
    rot[np.arange(half), np.arange(half) + half] = 1.0
    rot[np.arange(half, D), np.arange(half, D) - half] = -1.0

    ident = np.eye(D, dtype=np.float32)
    onesc = np.ones((D, D), dtype=np.float32)

    k = np.arange(D)[:, None, None]
    r = np.arange(4)[None, :, None]
    q = np.arange(SB)[None, None, :]
    masks = (r * D + k <= q).astype(np.float32)  # [128, 4, 512]
    return (
        cosT.astype(NPBF), sinT.astype(NPBF), rot.astype(NPBF),
        ident.astype(NPBF), onesc.astype(NPBF), masks.astype(NPBF),
    )


def _tile_x(xb):
    # [S, E] -> [NSB, 4, D, NEC//4, SB]: contiguous [128, 4, 512] DMA tiles,
    # element [g, qt, p, ne, s] = x[g*SB+s, (qt*4+ne)*D+p]
    a = np.asarray(xb, dtype=np.float32).reshape(NSB, SB, 4, NEC // 4, D)
    return np.ascontiguousarray(a.transpose(0, 2, 4, 3, 1)).astype(NPBF)


def _tile_w(w):
    # [E, M] -> [D, NEC, M]: element [p, ne, m] = w[ne*D+p, m]
    a = np.asarray(w, dtype=np.float32).reshape(NEC, D, -1)
    return np.ascontiguousarray(a.transpose(1, 0, 2)).astype(NPBF)


def build_in_maps(x, Wq, Wk, Wv, Wo):
    cosT, sinT, rot, ident, onesc, masks = _host_tables()
    in_maps = []
    for c in range(8):
        b, r = c // 4, c % 4
        in_maps.append(
            {
                "xT": _tile_x(x[b]),
                "wq": np.ascontiguousarray(
                    Wq[:, r * HQ * D : (r + 1) * HQ * D]
                    .astype(np.float32)
                    .reshape(2, NEC // 2, D, HQ, D)
                    .transpose(3, 0, 2, 1, 4)
                ).astype(NPBF),
                "wk": _tile_w(Wk[:, r * D : (r + 1) * D]),
                "wv": _tile_w(Wv[:, r * D : (r + 1) * D]),
                "wo": np.ascontiguousarray(
                    Wo[r * HQ * D : (r + 1) * HQ * D, :]
                    .astype(np.float32)
                    .reshape(HQ, D, E)
                    .transpose(1, 0, 2)
                ).astype(NPBF),
                "cosT": cosT,
                "sinT": sinT,
                "rot": rot,
                "ident": ident,
                "onesc": onesc,
                "masks": masks,
            }
        )

    return in_maps


def kernel(x, Wq, Wk, Wv, Wo):
    assert x.shape == (2, S, E)
    nc = _get_nc()
    in_maps = build_in_maps(x, Wq, Wk, Wv, Wo)
    res = run_bass_kernel_spmd(nc, in_maps, list(range(8)))
    ys = []
    for b in range(2):
        acc = None
        for c in range(b * 4, b * 4 + 4):
            y = np.asarray(res.results[c]["out"]).astype(np.float32)
            y[S - SB :, :] += np.asarray(res.results[c]["out2"]).astype(np.float32)
            acc = y if acc is None else acc + y
        ys.append(acc)
    return np.stack(ys, axis=0).astype(np.float32)


# revision 36
# speedup vs baseline: 1.0085x; 1.0085x over previous
"""GQA multi-head self-attention (16 heads / 4 KV heads / head_dim 128) with
rotate-half RoPE, for B=2, S=2048, E=2048 fp32 inputs, on 8 NeuronCores.

Sharding: 8 cores = 2 batches x 4 tensor-parallel ranks. Each rank owns 4
query heads + 1 KV head (column slices of Wq/Wk/Wv) and the matching row
slice of Wo; per-rank partial outputs are summed on the host (the Wo
all-reduce), batches are concatenated.

v4 (bf16 + scheduling): all matmul operands bf16 (PSUM stays fp32) -- keeps
the PE at 1 col/cycle, enables fast weight loads, halves HBM traffic, and
runs diagonal score blocks at N=128 full rate. Rel err ~6e-3 (budget 2e-2).

Softmax row-sums avoid PE matmuls per block: exp blocks accumulate
elementwise into two SBUF accumulators (even blocks on DVE, odd on the
otherwise-idle GpSimd; the last block on DVE to shorten the end-of-head
chain), then one ones-matmul per accumulator reduces partitions.

Scheduling: the attention q-blocks run ascending, and every head carries
dependency-free PE filler so exp/softmax chain latency never idles the PE:
the g=3 Q projections are withheld from phase A and emitted one chain per
g=0 head (whose attention is tiny), and each completed q-block's output
projection is dripped between later score blocks with an allowance that
spreads it across the following q-block's heads. The final q-block's output
projection is split: heads 0-2 partials stream to `out` early, head 3's
contribution lands in `out2` and is added on the host, shrinking the
end-of-kernel drain.
"""

import sys

sys.path.insert(0, "/opt/trn_rl_repo")

from contextlib import ExitStack

import ml_dtypes
import numpy as np

import concourse.bacc as bacc
import concourse.tile as tile
from concourse import mybir
from concourse.bass_utils import run_bass_kernel_spmd

BF = mybir.dt.bfloat16
F32 = mybir.dt.float32
NPBF = ml_dtypes.bfloat16

S = 2048  # sequence length
E = 2048  # embed dim
D = 128  # head dim
HQ = 4  # query heads per core
SB = 512  # s-block (free-dim tile)
NSB = S // SB  # 4
NEC = E // D  # 16 contraction chunks
NSC = S // D  # 16 s-chunks
SCALE = 1.0 / float(np.sqrt(D))

_CACHED_NC = None


def _build_nc():
    nc = bacc.Bacc("TRN2", target_bir_lowering=False, debug=False)

    xT = nc.dram_tensor("xT", [NSB, 4, D, NEC // 4, SB], BF, kind="ExternalInput")
    wq = nc.dram_tensor("wq", [HQ, 2, D, NEC // 2, D], BF, kind="ExternalInput")
    wk = nc.dram_tensor("wk", [D, NEC, D], BF, kind="ExternalInput")
    wv = nc.dram_tensor("wv", [D, NEC, D], BF, kind="ExternalInput")
    wo = nc.dram_tensor("wo", [D, HQ, E], BF, kind="ExternalInput")
    cosT = nc.dram_tensor("cosT", [D, S], BF, kind="ExternalInput")
    sinT = nc.dram_tensor("sinT", [D, S], BF, kind="ExternalInput")
    rot = nc.dram_tensor("rot", [D, D], BF, kind="ExternalInput")
    ident = nc.dram_tensor("ident", [D, D], BF, kind="ExternalInput")
    onesc = nc.dram_tensor("onesc", [D, D], BF, kind="ExternalInput")
    masks = nc.dram_tensor("masks", [D, 4, SB], BF, kind="ExternalInput")
    out = nc.dram_tensor("out", [S, E], BF, kind="ExternalOutput")
    out2 = nc.dram_tensor("out2", [SB, E], BF, kind="ExternalOutput")

    with tile.TileContext(nc) as tc, ExitStack() as ctx:
        pers = ctx.enter_context(tc.tile_pool(name="pers", bufs=1))
        qts = [
            [
                pers.tile([D, SB], BF, tag=f"qt{h}_{g}", name=f"qt{h}_{g}")
                for g in range(NSB)
            ]
            for h in range(HQ)
        ]
        kts = [
            pers.tile([D, SB], BF, tag=f"kts{g}", name=f"kts{g}")
            for g in range(NSB)
        ]
        vsb = [
            pers.tile([D, SB // D, D], BF, tag=f"vsb{g}", name=f"vsb{g}")
            for g in range(NSB)
        ]

        ps_pool = ctx.enter_context(tc.tile_pool(name="ps", bufs=1, space="PSUM"))

        class _TagPool:
            def __init__(self, tag, bufs):
                self.tag, self.bufs, self.n = tag, bufs, 0

            def tile(self, shape, dtype, **kw):
                self.n += 1
                return ps_pool.tile(
                    shape, dtype, tag=self.tag, bufs=self.bufs,
                    name=f"{self.tag}_{self.n}",
                )

        psq_pool = pst_pool = _TagPool("st3", 3)
        pskv_pool = psa_pool = _TagPool("acc", 2)
        psr_pool = psl_pool = _TagPool("one", 1)
        pstr_pool = pso_pool = _TagPool("sm", 2)

        # persistent phase-B constants, DMA'd during phase A so the A->B
        # boundary has no DMA wait
        wo_pool = ctx.enter_context(tc.tile_pool(name="woP", bufs=1))
        wot = wo_pool.tile([D, HQ, E], BF)
        lin_pool = ctx.enter_context(tc.tile_pool(name="lin", bufs=1))
        onest = lin_pool.tile([D, D], BF, tag="onest")
        maskt = lin_pool.tile([D, 4, SB], BF, tag="maskt")

        # pools that phase A and the deferred g=3 Q chains both use
        xs_pool = ctx.enter_context(tc.tile_pool(name="xs", bufs=8))
        wA_pool = ctx.enter_context(tc.tile_pool(name="wA", bufs=1))
        ropet = ctx.enter_context(tc.tile_pool(name="ropet", bufs=2))

        # PE pre-warm: dependency-free dummy matmuls on never-written SBUF
        # keep the PE busy through the input DMA ramp so the HAM clock-gate
        # un-throttles before the first real chain and the ramp is hidden.
        dummy_pool = ctx.enter_context(tc.tile_pool(name="dummy", bufs=1))
        dumt = dummy_pool.tile([D, SB], BF, tag="dumt")
        nc.gpsimd.memset(dumt[:], 0.0)
        psd = psr_pool.tile([D, SB], F32)
        for _ in range(12):
            nc.tensor.matmul(psd[:], dumt[:, 0:D], dumt[:], start=True, stop=True)

        # ---- Phase A: QKV projections + RoPE + V transpose ----
        # First DMAs in: part of the K weights plus the first g=0 x tile so
        # the first matmul chain starts as early as possible, then the rest
        # of the x stream, tables and later weights.
        def load_x(g):
            tiles = []
            for qt in range(4):
                t = xs_pool.tile(
                    [D, NEC // 4, SB], BF, tag="xs", name=f"xs{g}_{qt}"
                )
                nc.sync.dma_start(t[:], xT[g, qt])
                tiles.append(t)
            return tiles

        def load_wq(h):
            halves = []
            for hf in range(2):
                t = wA_pool.tile(
                    [D, NEC // 2, D], BF, tag=f"wq{h}_{hf}", name=f"wq{h}_{hf}"
                )
                nc.sync.dma_start(t[:], wq[h, hf])
                halves.append(t)
            return halves

        wkt = wA_pool.tile([D, NEC, D], BF)
        nc.sync.dma_start(wkt[:, 0:4, :], wk[:, 0:4, :])
        xh0 = []
        t = xs_pool.tile([D, NEC // 4, SB], BF, tag="xs", name="xs0_0")
        nc.sync.dma_start(t[:], xT[0, 0])
        xh0.append(t)
        nc.sync.dma_start(wkt[:, 4:, :], wk[:, 4:, :])
        for qt in range(1, 4):
            t = xs_pool.tile([D, NEC // 4, SB], BF, tag="xs", name=f"xs0_{qt}")
            nc.sync.dma_start(t[:], xT[0, qt])
            xh0.append(t)
        wvt = wA_pool.tile([D, NEC, D], BF)
        nc.sync.dma_start(wvt[:], wv[:])
        xh1 = load_x(1)  # prefetch g=1's x so its chains never wait
        rott = wA_pool.tile([D, D], BF, tag="rott")
        nc.sync.dma_start(rott[:], rot[:])
        cost = wA_pool.tile([D, S], BF, tag="cost")
        nc.sync.dma_start(cost[:], cosT[:])
        sint = wA_pool.tile([D, S], BF, tag="sint")
        nc.sync.dma_start(sint[:], sinT[:])
        wqh = [load_wq(h) for h in range(HQ)]
        idt = wA_pool.tile([D, D], BF, tag="idt")
        nc.sync.dma_start(idt[:], ident[:])
        # phase-B constants, early
        nc.sync.dma_start(onest[:], onesc[:])
        nc.sync.dma_start(maskt[:], masks[:])
        nc.sync.dma_start(wot[:], wo[:])

        def rope_store(src_ps, dst_slice, scale):
            # qc = rounded bf16 copy of the projection (folds 1/sqrt(D))
            qc = ropet.tile([D, SB], BF, tag="qc")
            nc.scalar.activation(
                qc[:], src_ps[:], mybir.ActivationFunctionType.Copy,
                scale=scale,
            )
            # pr = signed rotate-half via PE permutation matmul
            pr = psr_pool.tile([D, SB], F32)
            nc.tensor.matmul(pr[:], rott[:], qc[:], start=True, stop=True)
            tm = ropet.tile([D, SB], BF, tag="tm")
            nc.vector.tensor_mul(tm[:], qc[:], cost[:, dst_slice])
            tr = ropet.tile([D, SB], F32, tag="tr")
            nc.vector.tensor_mul(tr[:], pr[:], sint[:, dst_slice])
            return qc, tm, tr

        def emit_q(h, g, xh):
            def xc(e):
                return xh[e // (NEC // 4)][:, e % (NEC // 4), :]

            psq = psq_pool.tile([D, SB], F32)
            for e in range(NEC):
                nc.tensor.matmul(
                    psq[:],
                    wqh[h][e // (NEC // 2)][:, e % (NEC // 2), :],
                    xc(e),
                    start=(e == 0),
                    stop=(e == NEC - 1),
                )
            _, tm, tr = rope_store(psq, slice(g * SB, (g + 1) * SB), SCALE)
            nc.vector.tensor_add(qts[h][g][:], tm[:], tr[:])

        xh3 = None
        for g in range(NSB):
            gsl = slice(g * SB, (g + 1) * SB)
            xh = (xh0, xh1, None, None)[g] if g < 2 else load_x(g)

            def xc(e):
                return xh[e // (NEC // 4)][:, e % (NEC // 4), :]

            psk = pskv_pool.tile([D, SB], F32)
            for e in range(NEC):
                nc.tensor.matmul(
                    psk[:], wkt[:, e, :], xc(e),
                    start=(e == 0), stop=(e == NEC - 1),
                )
            _, tm, tr = rope_store(psk, gsl, 1.0)
            nc.vector.tensor_add(kts[g][:], tm[:], tr[:])

            psv = pskv_pool.tile([D, SB], F32)
            for e in range(NEC):
                nc.tensor.matmul(
                    psv[:], wvt[:, e, :], xc(e),
                    start=(e == 0), stop=(e == NEC - 1),
                )
            vt = ropet.tile([D, SB], BF, tag="vt")
            nc.vector.tensor_copy(vt[:], psv[:])
            for c in range(SB // D):
                ptr = pstr_pool.tile([D, D], BF)
                nc.tensor.transpose(ptr[:], vt[:, c * D : (c + 1) * D], idt[:])
                nc.vector.tensor_copy(vsb[g][:, c, :], ptr[:])

            if g == NSB - 1:
                # withhold the g=3 Q projections: they become PE filler for
                # the filler-less g=0 attention heads
                xh3 = xh
            else:
                for h in range(HQ):
                    emit_q(h, g, xh)

        # ---- Phase B: attention (scores^T -> exp -> mask -> l, attn^T) ----
        atn_pool = ctx.enter_context(tc.tile_pool(name="atnP", bufs=1))
        atn = [
            [
                atn_pool.tile([D, SB], BF, tag=f"atn{h}_{g}", name=f"atn{h}_{g}")
                for g in range(NSB)
            ]
            for h in range(HQ)
        ]
        with (
            tc.tile_pool(name="ptp", bufs=4) as pt_pool,
            tc.tile_pool(name="accp", bufs=2) as acc_pool,
            tc.tile_pool(name="lbp", bufs=2) as lb_pool,
            tc.tile_pool(name="outs", bufs=8) as out_pool,
        ):
            # Output-projection work for one (sc, nb) pair over heads `hs`:
            # emitted as filler between attention blocks so these
            # dependency-free matmuls soak up PE bubbles while exp/mask
            # chains are in flight. Every 4th copy goes to the Scalar engine
            # to keep DVE below the PE's phase-B load.
            nco = [0]

            def emit_c(sc, nb, hs=tuple(range(HQ)), dst=None, row0=None,
                       copy_eng=None, pool=None):
                po = (pool or pso_pool).tile([D, SB], F32)
                for i, h in enumerate(hs):
                    nc.tensor.matmul(
                        po[:],
                        atn[h][sc // 4][:, (sc % 4) * D : (sc % 4 + 1) * D],
                        wot[:, h, nb * SB : (nb + 1) * SB],
                        start=(i == 0),
                        stop=(i == len(hs) - 1),
                    )
                nco[0] += 1
                ot = out_pool.tile([D, SB], BF, tag="ot", name=f"ot{nco[0]}")
                if copy_eng is None:
                    copy_eng = "s" if nco[0] % 4 == 0 else "v"
                if copy_eng == "s":
                    nc.scalar.copy(ot[:], po[:])
                else:
                    nc.vector.tensor_copy(ot[:], po[:])
                if dst is None:
                    dst, row0 = out, sc * D
                nc.sync.dma_start(
                    dst[row0 : row0 + D, nb * SB : (nb + 1) * SB], ot[:]
                )

            cqueue = []
            for g in range(NSB):
                nkb = 4 * (g + 1)
                for h in range(HQ):
                    last_head = g == NSB - 1 and h == HQ - 1
                    # spread remaining filler over this q-block's heads
                    allow = 0
                    if cqueue and not last_head:
                        hleft = HQ - h if g < NSB - 1 else HQ - 1 - h
                        allow = -(-len(cqueue) // max(hleft, 1))
                    dripped = 0
                    # g=0 heads have no output-projection filler yet: the
                    # withheld g=3 Q projection chain (on the otherwise-idle
                    # sm PSUM tag) interleaves with the score blocks instead
                    psq3 = pso_pool.tile([D, SB], F32) if g == 0 else None
                    pa = psa_pool.tile([D, SB], F32)
                    # dual softmax-denominator accumulators: even score blocks
                    # sum on DVE, odd blocks on GpSimd (except the last block,
                    # which goes to DVE/accA to shorten the end-of-head chain)
                    accA = acc_pool.tile([D, SB], BF, tag="accA")
                    accB = None
                    if g > 0:
                        accB = acc_pool.tile(
                            [D, SB], BF, tag="accB", name="accB"
                        )
                    pending = []

                    def consume(kb, pt, qo):
                        nc.tensor.matmul(
                            pa[:, qo:SB], vsb[kb // 4][:, kb % 4, :], pt[:, qo:SB],
                            start=(kb == 0), stop=(kb == nkb - 1),
                        )

                    for kb in range(nkb):
                        # Diagonal blocks: queries below kb*D are fully
                        # masked; shrink N to the live 128-multiple range.
                        r = kb - 4 * g
                        qo = 0 if r < 1 else r * D
                        ps = pst_pool.tile([D, SB], F32)
                        nc.tensor.matmul(
                            ps[:, qo:SB],
                            kts[kb // 4][:, (kb % 4) * D : (kb % 4 + 1) * D],
                            qts[h][g][:, qo:SB],
                            start=True,
                            stop=True,
                        )
                        pt = pt_pool.tile([D, SB], BF, tag="pt")
                        nc.scalar.activation(
                            pt[:, qo:SB], ps[:, qo:SB],
                            mybir.ActivationFunctionType.Exp,
                        )
                        if r >= 0:
                            # the causal mask is non-trivial only on the
                            # 128-wide diagonal query sub-tile; columns past
                            # it are all-ones, columns before are excluded
                            # by qo
                            dsl = slice(r * D, (r + 1) * D)
                            nc.vector.tensor_mul(
                                pt[:, dsl], pt[:, dsl], maskt[:, r, dsl]
                            )
                        # even blocks and the last two odd blocks sum on DVE
                        # (fast, so the denominator closes right after the
                        # exp pipeline); GpSimd takes only early odd blocks,
                        # finishing well before the head's pl matmul needs it
                        if kb == 0:
                            nc.vector.tensor_copy(accA[:], pt[:])
                        elif kb % 2 == 1 and kb <= nkb - 5:
                            if kb == 1:
                                nc.gpsimd.tensor_copy(accB[:], pt[:])
                            else:
                                nc.gpsimd.tensor_add(
                                    accB[:, qo:SB], accB[:, qo:SB], pt[:, qo:SB]
                                )
                        else:
                            nc.vector.tensor_add(
                                accA[:, qo:SB], accA[:, qo:SB], pt[:, qo:SB]
                            )
                        pending.append((kb, pt, qo))
                        # keep PE two score-blocks ahead of the exp pipeline
                        if len(pending) > 2:
                            consume(*pending.pop(0))
                        # drip output-projection filler between score blocks
                        if kb % 2 == 1 and dripped < allow and cqueue:
                            emit_c(*cqueue.pop(0))
                            dripped += 1
                        if last_head and kb % 2 == 1:
                            # stream heads 0-2 of the final q-block's output
                            # projection while head 3 finishes
                            i = kb // 2
                            emit_c(12 + i // 2, 2 * (i % 2), hs=(0, 1, 2),
                                   copy_eng="s")
                            emit_c(12 + i // 2, 2 * (i % 2) + 1, hs=(0, 1, 2),
                                   copy_eng="s")
                        if g == 0:
                            for e in range(kb * 4, kb * 4 + 4):
                                nc.tensor.matmul(
                                    psq3[:],
                                    wqh[h][e // (NEC // 2)][:, e % (NEC // 2), :],
                                    xh3[e // (NEC // 4)][:, e % (NEC // 4), :],
                                    start=(e == 0),
                                    stop=(e == NEC - 1),
                                )
                    for item in pending:
                        consume(*item)

                    # softmax denominator: reduce both accumulators'
                    # partitions with ones-matmuls (early-finishing
                    # accumulator first), then normalize
                    pl = psl_pool.tile([D, SB], F32)
                    if g > 0:
                        nc.tensor.matmul(
                            pl[:], onest[:], accB[:], start=True, stop=False
                        )
                        nc.tensor.matmul(
                            pl[:], onest[:], accA[:], start=False, stop=True
                        )
                    else:
                        nc.tensor.matmul(
                            pl[:], onest[:], accA[:], start=True, stop=True
                        )
                    lb = lb_pool.tile([D, SB], F32, tag="lb")
                    nc.vector.reciprocal_approx_fast(lb[:], pl[:])
                    nc.vector.tensor_mul(atn[h][g][:], pa[:], lb[:])

                    if g == 0:
                        # finish the interleaved g=3 Q chain: RoPE + store
                        _, tm3, tr3 = rope_store(
                            psq3, slice((NSB - 1) * SB, NSB * SB), SCALE
                        )
                        nc.vector.tensor_add(qts[h][NSB - 1][:], tm3[:], tr3[:])
                    else:
                        while dripped < allow and cqueue:
                            emit_c(*cqueue.pop(0))
                            dripped += 1
                if g < NSB - 1:
                    cqueue.extend(
                        (sc, nb)
                        for sc in range(4 * g, 4 * (g + 1))
                        for nb in range(E // SB)
                    )
            # any leftover filler
            while cqueue:
                emit_c(*cqueue.pop(0))
            # head 3 of the final q-block: single-matmul groups into out2,
            # copies alternating scalar/vector so the drain isn't DVE-bound
            # attention is over, so the score-pipeline PSUM pool is free:
            # alternating the singles' accumulators between the sm and st3
            # pools deepens the rotation and makes the drain copy-bound
            for i, (sc, nb) in enumerate(
                (sc, nb) for sc in range(12, 16) for nb in range(E // SB)
            ):
                emit_c(sc, nb, hs=(3,), dst=out2, row0=(sc - 12) * D,
                       copy_eng=("s" if i % 2 == 0 else "v"),
                       pool=(pst_pool if i % 2 == 0 else pso_pool))

    nc.finalize()
    return nc


def _get_nc():
    global _CACHED_NC
    if _CACHED_NC is None:
        _CACHED_NC = _build_nc()
    return _CACHED_NC


def _host_tables():
    inv_freq = 1.0 / (10000.0 ** (np.arange(0, D, 2, dtype=np.float64) / D))
    ang = np.arange(S, dtype=np.float64)[:, None] * inv_freq[None, :]  # [S, 64]
    cos_half = np.cos(ang).T.astype(np.float32)  # [64, S]
    sin_half = np.sin(ang).T.astype(np.float32)
    cosT = np.concatenate([cos_half, cos_half], axis=0)  # [128, S]
    sinT = np.concatenate([sin_half, sin_half], axis=0)

    rot = np.zeros((D, D), dtype=np.float32)  # lhsT of rotate-half
    half = D // 2
    rot[np.arange(half), np.arange(half) + half] = 1.0
    rot[np.arange(half, D), np.arange(half, D) - half] = -1.0

    ident = np.eye(D, dtype=np.float32)
    onesc = np.ones((D, D), dtype=np.float32)

    k = np.arange(D)[:, None, None]
    r = np.arange(4)[None, :, None]
    q = np.arange(SB)[None, None, :]
    masks = (r * D + k <= q).astype(np.float32)  # [128, 4, 512]
    return (
        cosT.astype(NPBF), sinT.astype(NPBF), rot.astype(NPBF),
        ident.astype(NPBF), onesc.astype(NPBF), masks.astype(NPBF),
    )


def _tile_x(xb):
    # [S, E] -> [NSB, 4, D, NEC//4, SB]: contiguous [128, 4, 512] DMA tiles,
    # element [g, qt, p, ne, s] = x[g*SB+s, (qt*4+ne)*D+p]
    a = np.asarray(xb, dtype=np.float32).reshape(NSB, SB, 4, NEC // 4, D)
    return np.ascontiguousarray(a.transpose(0, 2, 4, 3, 1)).astype(NPBF)


def _tile_w(w):
    # [E, M] -> [D, NEC, M]: element [p, ne, m] = w[ne*D+p, m]
    a = np.asarray(w, dtype=np.float32).reshape(NEC, D, -1)
    return np.ascontiguousarray(a.transpose(1, 0, 2)).astype(NPBF)


def build_in_maps(x, Wq, Wk, Wv, Wo):
    cosT, sinT, rot, ident, onesc, masks = _host_tables()
    in_maps = []
    for c in range(8):
        b, r = c // 4, c % 4
        in_maps.append(
            {
                "xT": _tile_x(x[b]),
                "wq": np.ascontiguousarray(
                    Wq[:, r * HQ * D : (r + 1) * HQ * D]
                    .astype(np.float32)
                    .reshape(2, NEC // 2, D, HQ, D)
                    .transpose(3, 0, 2, 1, 4)
                ).astype(NPBF),
                "wk": _tile_w(Wk[:, r * D : (r + 1) * D]),
                "wv": _tile_w(Wv[:, r * D : (r + 1) * D]),
                "wo": np.ascontiguousarray(
                    Wo[r * HQ * D : (r + 1) * HQ * D, :]
                    .astype(np.float32)
                    .reshape(HQ, D, E)
                    .transpose(1, 0, 2)
                ).astype(NPBF),
                "cosT": cosT,
                "sinT": sinT,
                "rot": rot,
                "ident": ident,
                "onesc": onesc,
                "masks": masks,
            }
        )

    return in_maps


def kernel(x, Wq, Wk, Wv, Wo):
    assert x.shape == (2, S, E)
    nc = _get_nc()
    in_maps = build_in_maps(x, Wq, Wk, Wv, Wo)
    res = run_bass_kernel_spmd(nc, in_maps, list(range(8)))
    ys = []
    for b in range(2):
        acc = None
        for c in range(b * 4, b * 4 + 4):
            y = np.asarray(res.results[c]["out"]).astype(np.float32)
            y[S - SB :, :] += np.asarray(res.results[c]["out2"]).astype(np.float32)
            acc = y if acc is None else acc + y
        ys.append(acc)
    return np.stack(ys, axis=0).astype(np.float32)


# revision 37
# speedup vs baseline: 1.0099x; 1.0013x over previous
"""GQA multi-head self-attention (16 heads / 4 KV heads / head_dim 128) with
rotate-half RoPE, for B=2, S=2048, E=2048 fp32 inputs, on 8 NeuronCores.

Sharding: 8 cores = 2 batches x 4 tensor-parallel ranks. Each rank owns 4
query heads + 1 KV head (column slices of Wq/Wk/Wv) and the matching row
slice of Wo; per-rank partial outputs are summed on the host (the Wo
all-reduce), batches are concatenated.

v4 (bf16 + scheduling): all matmul operands bf16 (PSUM stays fp32) -- keeps
the PE at 1 col/cycle, enables fast weight loads, halves HBM traffic, and
runs diagonal score blocks at N=128 full rate. Rel err ~6e-3 (budget 2e-2).

Softmax row-sums avoid PE matmuls per block: exp blocks accumulate
elementwise into two SBUF accumulators (even blocks on DVE, odd on the
otherwise-idle GpSimd; the last block on DVE to shorten the end-of-head
chain), then one ones-matmul per accumulator reduces partitions.

Scheduling: the attention q-blocks run ascending, and every head carries
dependency-free PE filler so exp/softmax chain latency never idles the PE:
the g=3 Q projections are withheld from phase A and emitted one chain per
g=0 head (whose attention is tiny), and each completed q-block's output
projection is dripped between later score blocks with an allowance that
spreads it across the following q-block's heads. The final q-block's output
projection is split: heads 0-2 partials stream to `out` early, head 3's
contribution lands in `out2` and is added on the host, shrinking the
end-of-kernel drain.
"""

import sys

sys.path.insert(0, "/opt/trn_rl_repo")

from contextlib import ExitStack

import ml_dtypes
import numpy as np

import concourse.bacc as bacc
import concourse.tile as tile
from concourse import mybir
from concourse.bass_utils import run_bass_kernel_spmd

BF = mybir.dt.bfloat16
F32 = mybir.dt.float32
NPBF = ml_dtypes.bfloat16

S = 2048  # sequence length
E = 2048  # embed dim
D = 128  # head dim
HQ = 4  # query heads per core
SB = 512  # s-block (free-dim tile)
NSB = S // SB  # 4
NEC = E // D  # 16 contraction chunks
NSC = S // D  # 16 s-chunks
SCALE = 1.0 / float(np.sqrt(D))

_CACHED_NC = None


def _build_nc():
    nc = bacc.Bacc("TRN2", target_bir_lowering=False, debug=False)

    xT = nc.dram_tensor("xT", [NSB, 4, D, NEC // 4, SB], BF, kind="ExternalInput")
    wq = nc.dram_tensor("wq", [HQ, 2, D, NEC // 2, D], BF, kind="ExternalInput")
    wk = nc.dram_tensor("wk", [D, NEC, D], BF, kind="ExternalInput")
    wv = nc.dram_tensor("wv", [D, NEC, D], BF, kind="ExternalInput")
    wo = nc.dram_tensor("wo", [D, HQ, E], BF, kind="ExternalInput")
    cosT = nc.dram_tensor("cosT", [D, S], BF, kind="ExternalInput")
    sinT = nc.dram_tensor("sinT", [D, S], BF, kind="ExternalInput")
    rot = nc.dram_tensor("rot", [D, D], BF, kind="ExternalInput")
    ident = nc.dram_tensor("ident", [D, D], BF, kind="ExternalInput")
    onesc = nc.dram_tensor("onesc", [D, D], BF, kind="ExternalInput")
    masks = nc.dram_tensor("masks", [D, 4, SB], BF, kind="ExternalInput")
    out = nc.dram_tensor("out", [S, E], BF, kind="ExternalOutput")
    out2 = nc.dram_tensor("out2", [SB, E], BF, kind="ExternalOutput")

    with tile.TileContext(nc) as tc, ExitStack() as ctx:
        pers = ctx.enter_context(tc.tile_pool(name="pers", bufs=1))
        qts = [
            [
                pers.tile([D, SB], BF, tag=f"qt{h}_{g}", name=f"qt{h}_{g}")
                for g in range(NSB)
            ]
            for h in range(HQ)
        ]
        kts = [
            pers.tile([D, SB], BF, tag=f"kts{g}", name=f"kts{g}")
            for g in range(NSB)
        ]
        vsb = [
            pers.tile([D, SB // D, D], BF, tag=f"vsb{g}", name=f"vsb{g}")
            for g in range(NSB)
        ]

        ps_pool = ctx.enter_context(tc.tile_pool(name="ps", bufs=1, space="PSUM"))

        class _TagPool:
            def __init__(self, tag, bufs):
                self.tag, self.bufs, self.n = tag, bufs, 0

            def tile(self, shape, dtype, **kw):
                self.n += 1
                return ps_pool.tile(
                    shape, dtype, tag=self.tag, bufs=self.bufs,
                    name=f"{self.tag}_{self.n}",
                )

        psq_pool = pst_pool = _TagPool("st3", 3)
        pskv_pool = psa_pool = _TagPool("acc", 2)
        psr_pool = psl_pool = _TagPool("one", 1)
        pstr_pool = pso_pool = _TagPool("sm", 2)

        # persistent phase-B constants, DMA'd during phase A so the A->B
        # boundary has no DMA wait
        wo_pool = ctx.enter_context(tc.tile_pool(name="woP", bufs=1))
        wot = wo_pool.tile([D, HQ, E], BF)
        lin_pool = ctx.enter_context(tc.tile_pool(name="lin", bufs=1))
        onest = lin_pool.tile([D, D], BF, tag="onest")
        maskt = lin_pool.tile([D, 4, SB], BF, tag="maskt")

        # pools that phase A and the deferred g=3 Q chains both use
        xs_pool = ctx.enter_context(tc.tile_pool(name="xs", bufs=8))
        wA_pool = ctx.enter_context(tc.tile_pool(name="wA", bufs=1))
        ropet = ctx.enter_context(tc.tile_pool(name="ropet", bufs=2))

        # PE pre-warm: dependency-free dummy matmuls on never-written SBUF
        # keep the PE busy through the input DMA ramp so the HAM clock-gate
        # un-throttles before the first real chain and the ramp is hidden.
        dummy_pool = ctx.enter_context(tc.tile_pool(name="dummy", bufs=1))
        dumt = dummy_pool.tile([D, SB], BF, tag="dumt")
        nc.gpsimd.memset(dumt[:], 0.0)
        psd = psr_pool.tile([D, SB], F32)
        for _ in range(15):
            nc.tensor.matmul(psd[:], dumt[:, 0:D], dumt[:], start=True, stop=True)

        # ---- Phase A: QKV projections + RoPE + V transpose ----
        # First DMAs in: part of the K weights plus the first g=0 x tile so
        # the first matmul chain starts as early as possible, then the rest
        # of the x stream, tables and later weights.
        def load_x(g):
            tiles = []
            for qt in range(4):
                t = xs_pool.tile(
                    [D, NEC // 4, SB], BF, tag="xs", name=f"xs{g}_{qt}"
                )
                nc.sync.dma_start(t[:], xT[g, qt])
                tiles.append(t)
            return tiles

        def load_wq(h):
            halves = []
            for hf in range(2):
                t = wA_pool.tile(
                    [D, NEC // 2, D], BF, tag=f"wq{h}_{hf}", name=f"wq{h}_{hf}"
                )
                nc.sync.dma_start(t[:], wq[h, hf])
                halves.append(t)
            return halves

        wkt = wA_pool.tile([D, NEC, D], BF)
        nc.sync.dma_start(wkt[:, 0:4, :], wk[:, 0:4, :])
        xh0 = []
        t = xs_pool.tile([D, NEC // 4, SB], BF, tag="xs", name="xs0_0")
        nc.sync.dma_start(t[:], xT[0, 0])
        xh0.append(t)
        nc.sync.dma_start(wkt[:, 4:, :], wk[:, 4:, :])
        for qt in range(1, 4):
            t = xs_pool.tile([D, NEC // 4, SB], BF, tag="xs", name=f"xs0_{qt}")
            nc.sync.dma_start(t[:], xT[0, qt])
            xh0.append(t)
        wvt = wA_pool.tile([D, NEC, D], BF)
        nc.sync.dma_start(wvt[:], wv[:])
        xh1 = load_x(1)  # prefetch g=1's x so its chains never wait
        rott = wA_pool.tile([D, D], BF, tag="rott")
        nc.sync.dma_start(rott[:], rot[:])
        cost = wA_pool.tile([D, S], BF, tag="cost")
        nc.sync.dma_start(cost[:], cosT[:])
        sint = wA_pool.tile([D, S], BF, tag="sint")
        nc.sync.dma_start(sint[:], sinT[:])
        wqh = [load_wq(h) for h in range(HQ)]
        idt = wA_pool.tile([D, D], BF, tag="idt")
        nc.sync.dma_start(idt[:], ident[:])
        # phase-B constants, early
        nc.sync.dma_start(onest[:], onesc[:])
        nc.sync.dma_start(maskt[:], masks[:])
        nc.sync.dma_start(wot[:], wo[:])

        def rope_store(src_ps, dst_slice, scale):
            # qc = rounded bf16 copy of the projection (folds 1/sqrt(D))
            qc = ropet.tile([D, SB], BF, tag="qc")
            nc.scalar.activation(
                qc[:], src_ps[:], mybir.ActivationFunctionType.Copy,
                scale=scale,
            )
            # pr = signed rotate-half via PE permutation matmul
            pr = psr_pool.tile([D, SB], F32)
            nc.tensor.matmul(pr[:], rott[:], qc[:], start=True, stop=True)
            tm = ropet.tile([D, SB], BF, tag="tm")
            nc.vector.tensor_mul(tm[:], qc[:], cost[:, dst_slice])
            tr = ropet.tile([D, SB], F32, tag="tr")
            nc.vector.tensor_mul(tr[:], pr[:], sint[:, dst_slice])
            return qc, tm, tr

        def emit_q(h, g, xh):
            def xc(e):
                return xh[e // (NEC // 4)][:, e % (NEC // 4), :]

            psq = psq_pool.tile([D, SB], F32)
            for e in range(NEC):
                nc.tensor.matmul(
                    psq[:],
                    wqh[h][e // (NEC // 2)][:, e % (NEC // 2), :],
                    xc(e),
                    start=(e == 0),
                    stop=(e == NEC - 1),
                )
            _, tm, tr = rope_store(psq, slice(g * SB, (g + 1) * SB), SCALE)
            nc.vector.tensor_add(qts[h][g][:], tm[:], tr[:])

        xh3 = None
        for g in range(NSB):
            gsl = slice(g * SB, (g + 1) * SB)
            xh = (xh0, xh1, None, None)[g] if g < 2 else load_x(g)

            def xc(e):
                return xh[e // (NEC // 4)][:, e % (NEC // 4), :]

            psk = pskv_pool.tile([D, SB], F32)
            for e in range(NEC):
                nc.tensor.matmul(
                    psk[:], wkt[:, e, :], xc(e),
                    start=(e == 0), stop=(e == NEC - 1),
                )
            _, tm, tr = rope_store(psk, gsl, 1.0)
            nc.vector.tensor_add(kts[g][:], tm[:], tr[:])

            psv = pskv_pool.tile([D, SB], F32)
            for e in range(NEC):
                nc.tensor.matmul(
                    psv[:], wvt[:, e, :], xc(e),
                    start=(e == 0), stop=(e == NEC - 1),
                )
            vt = ropet.tile([D, SB], BF, tag="vt")
            nc.vector.tensor_copy(vt[:], psv[:])
            for c in range(SB // D):
                ptr = pstr_pool.tile([D, D], BF)
                nc.tensor.transpose(ptr[:], vt[:, c * D : (c + 1) * D], idt[:])
                nc.vector.tensor_copy(vsb[g][:, c, :], ptr[:])

            if g == NSB - 1:
                # withhold the g=3 Q projections: they become PE filler for
                # the filler-less g=0 attention heads
                xh3 = xh
            else:
                for h in range(HQ):
                    emit_q(h, g, xh)

        # ---- Phase B: attention (scores^T -> exp -> mask -> l, attn^T) ----
        atn_pool = ctx.enter_context(tc.tile_pool(name="atnP", bufs=1))
        atn = [
            [
                atn_pool.tile([D, SB], BF, tag=f"atn{h}_{g}", name=f"atn{h}_{g}")
                for g in range(NSB)
            ]
            for h in range(HQ)
        ]
        with (
            tc.tile_pool(name="ptp", bufs=4) as pt_pool,
            tc.tile_pool(name="accp", bufs=2) as acc_pool,
            tc.tile_pool(name="lbp", bufs=2) as lb_pool,
            tc.tile_pool(name="outs", bufs=8) as out_pool,
        ):
            # Output-projection work for one (sc, nb) pair over heads `hs`:
            # emitted as filler between attention blocks so these
            # dependency-free matmuls soak up PE bubbles while exp/mask
            # chains are in flight. Every 4th copy goes to the Scalar engine
            # to keep DVE below the PE's phase-B load.
            nco = [0]

            def emit_c(sc, nb, hs=tuple(range(HQ)), dst=None, row0=None,
                       copy_eng=None, pool=None):
                po = (pool or pso_pool).tile([D, SB], F32)
                for i, h in enumerate(hs):
                    nc.tensor.matmul(
                        po[:],
                        atn[h][sc // 4][:, (sc % 4) * D : (sc % 4 + 1) * D],
                        wot[:, h, nb * SB : (nb + 1) * SB],
                        start=(i == 0),
                        stop=(i == len(hs) - 1),
                    )
                nco[0] += 1
                ot = out_pool.tile([D, SB], BF, tag="ot", name=f"ot{nco[0]}")
                if copy_eng is None:
                    copy_eng = "s" if nco[0] % 4 == 0 else "v"
                if copy_eng == "s":
                    nc.scalar.copy(ot[:], po[:])
                else:
                    nc.vector.tensor_copy(ot[:], po[:])
                if dst is None:
                    dst, row0 = out, sc * D
                nc.sync.dma_start(
                    dst[row0 : row0 + D, nb * SB : (nb + 1) * SB], ot[:]
                )

            cqueue = []
            for g in range(NSB):
                nkb = 4 * (g + 1)
                for h in range(HQ):
                    last_head = g == NSB - 1 and h == HQ - 1
                    # spread remaining filler over this q-block's heads
                    allow = 0
                    if cqueue and not last_head:
                        hleft = HQ - h if g < NSB - 1 else HQ - 1 - h
                        allow = -(-len(cqueue) // max(hleft, 1))
                    dripped = 0
                    # g=0 heads have no output-projection filler yet: the
                    # withheld g=3 Q projection chain (on the otherwise-idle
                    # sm PSUM tag) interleaves with the score blocks instead
                    psq3 = pso_pool.tile([D, SB], F32) if g == 0 else None
                    pa = psa_pool.tile([D, SB], F32)
                    # dual softmax-denominator accumulators: even score blocks
                    # sum on DVE, odd blocks on GpSimd (except the last block,
                    # which goes to DVE/accA to shorten the end-of-head chain)
                    accA = acc_pool.tile([D, SB], BF, tag="accA")
                    accB = None
                    if g > 0:
                        accB = acc_pool.tile(
                            [D, SB], BF, tag="accB", name="accB"
                        )
                    pending = []

                    def consume(kb, pt, qo):
                        nc.tensor.matmul(
                            pa[:, qo:SB], vsb[kb // 4][:, kb % 4, :], pt[:, qo:SB],
                            start=(kb == 0), stop=(kb == nkb - 1),
                        )

                    for kb in range(nkb):
                        # Diagonal blocks: queries below kb*D are fully
                        # masked; shrink N to the live 128-multiple range.
                        r = kb - 4 * g
                        qo = 0 if r < 1 else r * D
                        ps = pst_pool.tile([D, SB], F32)
                        nc.tensor.matmul(
                            ps[:, qo:SB],
                            kts[kb // 4][:, (kb % 4) * D : (kb % 4 + 1) * D],
                            qts[h][g][:, qo:SB],
                            start=True,
                            stop=True,
                        )
                        pt = pt_pool.tile([D, SB], BF, tag="pt")
                        nc.scalar.activation(
                            pt[:, qo:SB], ps[:, qo:SB],
                            mybir.ActivationFunctionType.Exp,
                        )
                        if r >= 0:
                            # the causal mask is non-trivial only on the
                            # 128-wide diagonal query sub-tile; columns past
                            # it are all-ones, columns before are excluded
                            # by qo
                            dsl = slice(r * D, (r + 1) * D)
                            nc.vector.tensor_mul(
                                pt[:, dsl], pt[:, dsl], maskt[:, r, dsl]
                            )
                        # even blocks and the last two odd blocks sum on DVE
                        # (fast, so the denominator closes right after the
                        # exp pipeline); GpSimd takes only early odd blocks,
                        # finishing well before the head's pl matmul needs it
                        if kb == 0:
                            nc.vector.tensor_copy(accA[:], pt[:])
                        elif kb % 2 == 1 and kb <= nkb - 5:
                            if kb == 1:
                                nc.gpsimd.tensor_copy(accB[:], pt[:])
                            else:
                                nc.gpsimd.tensor_add(
                                    accB[:, qo:SB], accB[:, qo:SB], pt[:, qo:SB]
                                )
                        else:
                            nc.vector.tensor_add(
                                accA[:, qo:SB], accA[:, qo:SB], pt[:, qo:SB]
                            )
                        pending.append((kb, pt, qo))
                        # keep PE two score-blocks ahead of the exp pipeline
                        if len(pending) > 2:
                            consume(*pending.pop(0))
                        # drip output-projection filler between score blocks
                        if kb % 2 == 1 and dripped < allow and cqueue:
                            emit_c(*cqueue.pop(0))
                            dripped += 1
                        if last_head and kb % 2 == 1:
                            # stream heads 0-2 of the final q-block's output
                            # projection while head 3 finishes
                            i = kb // 2
                            emit_c(12 + i // 2, 2 * (i % 2), hs=(0, 1, 2),
                                   copy_eng="s")
                            emit_c(12 + i // 2, 2 * (i % 2) + 1, hs=(0, 1, 2),
                                   copy_eng="s")
                        if g == 0:
                            for e in range(kb * 4, kb * 4 + 4):
                                nc.tensor.matmul(
                                    psq3[:],
                                    wqh[h][e // (NEC // 2)][:, e % (NEC // 2), :],
                                    xh3[e // (NEC // 4)][:, e % (NEC // 4), :],
                                    start=(e == 0),
                                    stop=(e == NEC - 1),
                                )
                    for item in pending:
                        consume(*item)

                    # softmax denominator: reduce both accumulators'
                    # partitions with ones-matmuls (early-finishing
                    # accumulator first), then normalize
                    pl = psl_pool.tile([D, SB], F32)
                    if g > 0:
                        nc.tensor.matmul(
                            pl[:], onest[:], accB[:], start=True, stop=False
                        )
                        nc.tensor.matmul(
                            pl[:], onest[:], accA[:], start=False, stop=True
                        )
                    else:
                        nc.tensor.matmul(
                            pl[:], onest[:], accA[:], start=True, stop=True
                        )
                    lb = lb_pool.tile([D, SB], F32, tag="lb")
                    nc.vector.reciprocal_approx_fast(lb[:], pl[:])
                    nc.vector.tensor_mul(atn[h][g][:], pa[:], lb[:])

                    if g == 0:
                        # finish the interleaved g=3 Q chain: RoPE + store
                        _, tm3, tr3 = rope_store(
                            psq3, slice((NSB - 1) * SB, NSB * SB), SCALE
                        )
                        nc.vector.tensor_add(qts[h][NSB - 1][:], tm3[:], tr3[:])
                    else:
                        while dripped < allow and cqueue:
                            emit_c(*cqueue.pop(0))
                            dripped += 1
                if g < NSB - 1:
                    cqueue.extend(
                        (sc, nb)
                        for sc in range(4 * g, 4 * (g + 1))
                        for nb in range(E // SB)
                    )
            # any leftover filler
            while cqueue:
                emit_c(*cqueue.pop(0))
            # head 3 of the final q-block: single-matmul groups into out2,
            # copies alternating scalar/vector so the drain isn't DVE-bound
            # attention is over, so the score-pipeline PSUM pool is free:
            # alternating the singles' accumulators between the sm and st3
            # pools deepens the rotation and makes the drain copy-bound
            for i, (sc, nb) in enumerate(
                (sc, nb) for sc in range(12, 16) for nb in range(E // SB)
            ):
                emit_c(sc, nb, hs=(3,), dst=out2, row0=(sc - 12) * D,
                       copy_eng=("s" if i % 2 == 0 else "v"),
                       pool=(pst_pool if i % 2 == 0 else pso_pool))

    nc.finalize()
    return nc


def _get_nc():
    global _CACHED_NC
    if _CACHED_NC is None:
        _CACHED_NC = _build_nc()
    return _CACHED_NC


def _host_tables():
    inv_freq = 1.0 / (10000.0 ** (np.arange(0, D, 2, dtype=np.float64) / D))
    ang = np.arange(S, dtype=np.float64)[:, None] * inv_freq[None, :]  # [S, 64]
    cos_half = np.cos(ang).T.astype(np.float32)  # [64, S]
    sin_half = np.sin(ang).T.astype(np.float32)
    cosT = np.concatenate([cos_half, cos_half], axis=0)  # [128, S]
    sinT = np.concatenate([sin_half, sin_half], axis=0)

    rot = np.zeros((D, D), dtype=np.float32)  # lhsT of rotate-half
    half = D // 2
    rot[np.arange(half), np.arange(half) + half] = 1.0
    rot[np.arange(half, D), np.arange(half, D) - half] = -1.0

    ident = np.eye(D, dtype=np.float32)
    onesc = np.ones((D, D), dtype=np.float32)

    k = np.arange(D)[:, None, None]
    r = np.arange(4)[None, :, None]
    q = np.arange(SB)[None, None, :]
    masks = (r * D + k <= q).astype(np.float32)  # [128, 4, 512]
    return (
        cosT.astype(NPBF), sinT.astype(NPBF), rot.astype(NPBF),
        ident.astype(NPBF), onesc.astype(NPBF), masks.astype(NPBF),
    )


def _tile_x(xb):
    # [S, E] -> [NSB, 4, D, NEC//4, SB]: contiguous [128, 4, 512] DMA tiles,
    # element [g, qt, p, ne, s] = x[g*SB+s, (qt*4+ne)*D+p]
    a = np.asarray(xb, dtype=np.float32).reshape(NSB, SB, 4, NEC // 4, D)
    return np.ascontiguousarray(a.transpose(0, 2, 4, 3, 1)).astype(NPBF)


def _tile_w(w):
    # [E, M] -> [D, NEC, M]: element [p, ne, m] = w[ne*D+p, m]
    a = np.asarray(w, dtype=np.float32).reshape(NEC, D, -1)
    return np.ascontiguousarray(a.transpose(1, 0, 2)).astype(NPBF)


def build_in_maps(x, Wq, Wk, Wv, Wo):
    cosT, sinT, rot, ident, onesc, masks = _host_tables()
    in_maps = []
    for c in range(8):
        b, r = c // 4, c % 4
        in_maps.append(
            {
                "xT": _tile_x(x[b]),
                "wq": np.ascontiguousarray(
                    Wq[:, r * HQ * D : (r + 1) * HQ * D]
                    .astype(np.float32)
                    .reshape(2, NEC // 2, D, HQ, D)
                    .transpose(3, 0, 2, 1, 4)
                ).astype(NPBF),
                "wk": _tile_w(Wk[:, r * D : (r + 1) * D]),
                "wv": _tile_w(Wv[:, r * D : (r + 1) * D]),
                "wo": np.ascontiguousarray(
                    Wo[r * HQ * D : (r + 1) * HQ * D, :]
                    .astype(np.float32)
                    .reshape(HQ, D, E)
                    .transpose(1, 0, 2)
                ).astype(NPBF),
                "cosT": cosT,
                "sinT": sinT,
                "rot": rot,
                "ident": ident,
                "onesc": onesc,
                "masks": masks,
            }
        )

    return in_maps


def kernel(x, Wq, Wk, Wv, Wo):
    assert x.shape == (2, S, E)
    nc = _get_nc()
    in_maps = build_in_maps(x, Wq, Wk, Wv, Wo)
    res = run_bass_kernel_spmd(nc, in_maps, list(range(8)))
    ys = []
    for b in range(2):
        acc = None
        for c in range(b * 4, b * 4 + 4):
            y = np.asarray(res.results[c]["out"]).astype(np.float32)
            y[S - SB :, :] += np.asarray(res.results[c]["out2"]).astype(np.float32)
            acc = y if acc is None else acc + y
        ys.append(acc)
    return np.stack(ys, axis=0).astype(np.float32)
